# revision 14
# baseline (speedup 1.0000x reference)
"""MADPSNet MoE-routing kernel for 8 Trainium2 NeuronCores.

The reference computes every expert on the full stacked input and then
gathers one expert per agent.  The routing indices (laac_shallow /
laac_deep) are host-visible numpy values, so we do the routing on the
host: per agent we select the 4 weight matrices of its chosen experts
and run only the selected chain

    x[2048,256] @ W1[256,512] -> relu -> @ W2[512,256] -> relu
                -> @ W3[256,512] -> relu -> @ W4[512,128] (+bias)

One agent per NeuronCore (A == 8 == n_cores), no collectives.

Layout: everything feature-major on chip (features on the 128
partitions, batch on the free dim).  The host pre-packs

    x   [128, 2*2048]  col = k*2048 + b, row p  <=>  x[b, k*128+p]
    wN  [128, K/128*M] col = k*M + j           <=>  W[k*128+p, j]
    bias[128, 11]      col j = 128-chunk j of [b1(4) b2(2) b3(4) b4(1)]

so every DMA is a single large contiguous transfer.  Matmuls run in
float32r (full PE rate for moving dim >= 256, ~fp32 accuracy), PSUM
accumulates fp32, bias+relu runs alternately on ScalarE / VectorE.
The kernel returns out^T [128, 2048] per core; host transposes back.
"""

import os

import numpy as np

import concourse.bass as bass
import concourse.mybir as mybir
from concourse import bacc
from concourse.bass_utils import run_bass_kernel_spmd
from concourse.tile import TileContext

A, B, S = 8, 2048, 256
H1, H2, D1, D2 = 512, 256, 512, 128
P = 128
BT = 512            # batch tile (psum bank: 512 fp32)
NBT = B // BT

_DT_MAP = {
    "f32": mybir.dt.float32,
    "f32r": mybir.dt.float32r,
    "bf16": mybir.dt.bfloat16,
}

# layer: (k_chunks, m_chunks, bias col offset, relu?)
_LAYERS = [
    (S // P, H1 // P, 0, True),    # L1: 256 -> 512
    (H1 // P, H2 // P, 4, True),   # L2: 512 -> 256
    (H2 // P, D1 // P, 6, True),   # L3: 256 -> 512
    (D1 // P, D2 // P, 10, False), # L4: 512 -> 128
]


def _build(dt_name: str, add_bias: bool) -> bass.Bass:
    dt = _DT_MAP[dt_name]
    f32 = mybir.dt.float32
    nc = bacc.Bacc(None, target_bir_lowering=False, debug=False)

    x_d = nc.dram_tensor("x", [P, (S // P) * B], dt, kind="ExternalInput")
    w_ds = [
        nc.dram_tensor("w1", [P, (S // P) * H1], dt, kind="ExternalInput"),
        nc.dram_tensor("w2", [P, (H1 // P) * H2], dt, kind="ExternalInput"),
        nc.dram_tensor("w3", [P, (H2 // P) * D1], dt, kind="ExternalInput"),
        nc.dram_tensor("w4", [P, (D1 // P) * D2], dt, kind="ExternalInput"),
    ]
    b_d = (
        nc.dram_tensor("bias", [P, 11], f32, kind="ExternalInput")
        if add_bias
        else None
    )
    out_d = nc.dram_tensor("out", [D2, B], f32, kind="ExternalOutput")

    with TileContext(nc) as tc:
        with (
            tc.tile_pool(name="persist", bufs=1) as pp,
            tc.tile_pool(name="psum", bufs=8, space="PSUM") as psp,
        ):
            xt = pp.tile([P, (S // P) * B], dt, tag="xt", name="xt")
            wts = [
                pp.tile(
                    [P, w_ds[i].shape[1]], dt, tag=f"w{i}", name=f"w{i}_sb"
                )
                for i in range(4)
            ]
            bti = (
                pp.tile([P, 11], f32, tag="bias", name="bias_sb")
                if add_bias
                else None
            )
            scr = (
                pp.tile([P, 2], f32, tag="scr", name="scr") if add_bias else None
            )
            acts = [
                [
                    pp.tile([P, B], dt, tag=f"a{li}_{i}", name=f"a{li}_{i}")
                    for i in range(n)
                ]
                for li, n in [(1, H1 // P), (2, H2 // P), (3, D1 // P)]
            ]
            acts.append([pp.tile([P, B], f32, tag="ot", name="ot")])

            # x DMA'd per (k-chunk, batch-tile) so L1 can start early
            for k in range(S // P):
                for bt in range(NBT):
                    sl = slice(k * B + bt * BT, k * B + (bt + 1) * BT)
                    nc.sync.dma_start(xt[:, sl], x_d[:, sl])
            for i in range(4):
                nc.sync.dma_start(wts[i][:], w_ds[i][:])
            if add_bias:
                nc.sync.dma_start(bti[:], b_d[:])
                # advance ACT/DVE engine clocks past the bias DMA so the
                # real post-matmul ops carry a single (PE) wait each — the
                # AC/DVE instruction structs have very few wait slots.
                nc.scalar.copy(scr[:, 0:1], bti[:, 0:1])
                nc.vector.tensor_copy(scr[:, 1:2], bti[:, 0:1])

            xch = [xt[:, k * B : (k + 1) * B] for k in range(S // P)]
            srcs = xch
            for li, (kc, mc, boff, relu) in enumerate(_LAYERS):
                M = mc * P
                wt = wts[li]
                dsts = acts[li]
                for m in range(mc):
                    # fixed engine per dst tile: one writer engine per tile
                    use_act = (li < 3) and (m < mc // 2 or mc == 1)
                    for bt in range(NBT):
                        ps = psp.tile([P, BT], f32, tag="ps", name="ps")
                        for k in range(kc):
                            nc.tensor.matmul(
                                ps[:],
                                wt[:, k * M + m * P : k * M + (m + 1) * P],
                                srcs[k][:, bt * BT : (bt + 1) * BT],
                                start=(k == 0),
                                stop=(k == kc - 1),
                            )
                        dst = dsts[m][:, bt * BT : (bt + 1) * BT]
                        if add_bias:
                            bias_ap = bti[:, boff + m : boff + m + 1]
                            if use_act:
                                func = (
                                    mybir.ActivationFunctionType.Relu
                                    if relu
                                    else mybir.ActivationFunctionType.Identity
                                )
                                nc.scalar.activation(
                                    dst, ps[:], func, bias=bias_ap
                                )
                            elif relu:
                                nc.vector.tensor_scalar(
                                    dst,
                                    ps[:],
                                    bias_ap,
                                    0.0,
                                    mybir.AluOpType.add,
                                    mybir.AluOpType.max,
                                )
                            else:
                                nc.vector.tensor_scalar_add(dst, ps[:], bias_ap)
                        elif use_act:
                            func = (
                                mybir.ActivationFunctionType.Relu
                                if relu
                                else mybir.ActivationFunctionType.Copy
                            )
                            nc.scalar.activation(dst, ps[:], func)
                        elif relu:
                            nc.vector.tensor_scalar_max(dst, ps[:], 0.0)
                        else:
                            nc.vector.tensor_copy(dst, ps[:])
                        if li == 3:
                            nc.sync.dma_start(
                                out_d[:, bt * BT : (bt + 1) * BT], dst
                            )
                srcs = dsts
    nc.compile()
    return nc


_BUILT: dict[tuple[str, bool], bass.Bass] = {}


def _get_nc(dt_name: str, add_bias: bool = False) -> bass.Bass:
    key = (dt_name, add_bias)
    if key not in _BUILT:
        _BUILT[key] = _build(dt_name, add_bias)
    return _BUILT[key]


def _np_dt(dt_name: str):
    if dt_name == "bf16":
        import ml_dtypes

        return ml_dtypes.bfloat16
    return np.float32


def _packw(w: np.ndarray, np_dt) -> np.ndarray:
    k, m = w.shape
    return np.ascontiguousarray(
        w.reshape(k // P, P, m).transpose(1, 0, 2).reshape(P, -1).astype(np_dt)
    )


def _prepare(inputs, dt_name):
    """Returns (add_bias, in_maps) for run_bass_kernel_spmd."""
    np_dt = _np_dt(dt_name)

    x = np.asarray(inputs["inputs"], dtype=np.float32)
    sel_s = np.asarray(inputs["laac_shallow"]).reshape(-1).astype(np.int64)
    sel_d = np.asarray(inputs["laac_deep"]).reshape(-1).astype(np.int64)
    Ws1 = np.asarray(inputs["Ws1"], dtype=np.float32)
    Ws2 = np.asarray(inputs["Ws2"], dtype=np.float32)
    Wd1 = np.asarray(inputs["Wd1"], dtype=np.float32)
    Wd2 = np.asarray(inputs["Wd2"], dtype=np.float32)
    bs1 = np.asarray(inputs["bs1"], dtype=np.float32)
    bs2 = np.asarray(inputs["bs2"], dtype=np.float32)
    bd1 = np.asarray(inputs["bd1"], dtype=np.float32)
    bd2 = np.asarray(inputs["bd2"], dtype=np.float32)

    add_bias = any(
        float(np.abs(b).max()) != 0.0 for b in (bs1, bs2, bd1, bd2)
    )

    in_maps = []
    for a in range(A):
        es, ed = int(sel_s[a]), int(sel_d[a])
        xp = np.ascontiguousarray(
            x[a].reshape(B, S // P, P).transpose(2, 1, 0).reshape(P, -1).astype(np_dt)
        )
        m = {
            "x": xp,
            "w1": _packw(Ws1[es], np_dt),
            "w2": _packw(Ws2[es], np_dt),
            "w3": _packw(Wd1[ed], np_dt),
            "w4": _packw(Wd2[ed], np_dt),
        }
        if add_bias:
            bias_cols = np.concatenate([bs1[es], bs2[es], bd1[ed], bd2[ed]])
            m["bias"] = np.ascontiguousarray(
                bias_cols.reshape(11, P).T, dtype=np.float32
            )
        in_maps.append(m)
    return add_bias, in_maps


def kernel(**inputs) -> np.ndarray:
    dt_name = os.environ.get("MADPS_DT", "f32r")
    add_bias, in_maps = _prepare(inputs, dt_name)
    nc = _get_nc(dt_name, add_bias)
    res = run_bass_kernel_spmd(nc, in_maps, list(range(A)))
    out = np.stack([np.asarray(res.results[a]["out"]).T for a in range(A)])
    return np.ascontiguousarray(out.astype(np.float32))


# revision 16
# speedup vs baseline: 1.0756x; 1.0756x over previous
"""MADPSNet MoE-routing kernel for 8 Trainium2 NeuronCores.

The reference computes every expert on the full stacked input and then
gathers one expert per agent.  The routing indices (laac_shallow /
laac_deep) are host-visible numpy values, so we do the routing on the
host: per agent we select the 4 weight matrices of its chosen experts
and run only the selected chain

    x[2048,256] @ W1[256,512] -> relu -> @ W2[512,256] -> relu
                -> @ W3[256,512] -> relu -> @ W4[512,128] (+bias)

One agent per NeuronCore (A == 8 == n_cores), no collectives.

Layout: everything feature-major on chip (features on the 128
partitions, batch on the free dim).  The host pre-packs

    x   [128, 2*2048]   col = k*2048 + b      <=>  x[b, k*128+p]
    wN  [128, K/128*M]  col = (m*kc + k)*128+j <=> W[k*128+p, m*128+j]
    bias[128, 11]       col j = 128-chunk j of [b1(4) b2(2) b3(4) b4(1)]

so every DMA is a large contiguous transfer, issued in the order the
compute needs it (w1 chunk 0 / x batch-tile 0 first).  Matmuls run in
float32r (full PE rate for moving dim >= 256, ~fp32 accuracy), PSUM
accumulates fp32, bias+relu runs split across ScalarE / VectorE with a
fixed engine per destination tile.  A few warm-up matmuls on a zeroed
scratch tile keep the PE busy from kernel start so the HAM clock
un-throttles (1.2 -> 2.4 GHz) before the real work arrives.
The kernel returns out^T [128, 2048] per core; the host transposes.
"""

import os

import numpy as np

import concourse.bass as bass
import concourse.mybir as mybir
from concourse import bacc
from concourse.bass_utils import run_bass_kernel_spmd
from concourse.tile import TileContext

A, B, S = 8, 2048, 256
H1, H2, D1, D2 = 512, 256, 512, 128
P = 128
BT = 512            # batch tile (psum bank: 512 fp32)
NBT = B // BT

_DT_MAP = {
    "f32": mybir.dt.float32,
    "f32r": mybir.dt.float32r,
    "bf16": mybir.dt.bfloat16,
}

# layer: (k_chunks, m_chunks, bias col offset, relu?)
_LAYERS = [
    (S // P, H1 // P, 0, True),    # L1: 256 -> 512
    (H1 // P, H2 // P, 4, True),   # L2: 512 -> 256
    (H2 // P, D1 // P, 6, True),   # L3: 256 -> 512
    (D1 // P, D2 // P, 10, False), # L4: 512 -> 128
]


def _build(dt_name: str, add_bias: bool, warm: int) -> bass.Bass:
    dt = _DT_MAP[dt_name]
    f32 = mybir.dt.float32
    nc = bacc.Bacc(None, target_bir_lowering=False, debug=False)

    x_d = nc.dram_tensor("x", [P, (S // P) * B], dt, kind="ExternalInput")
    w_ds = [
        nc.dram_tensor("w1", [P, (S // P) * H1], dt, kind="ExternalInput"),
        nc.dram_tensor("w2", [P, (H1 // P) * H2], dt, kind="ExternalInput"),
        nc.dram_tensor("w3", [P, (H2 // P) * D1], dt, kind="ExternalInput"),
        nc.dram_tensor("w4", [P, (D1 // P) * D2], dt, kind="ExternalInput"),
    ]
    b_d = (
        nc.dram_tensor("bias", [P, 11], f32, kind="ExternalInput")
        if add_bias
        else None
    )
    out_d = nc.dram_tensor("out", [D2, B], f32, kind="ExternalOutput")

    with TileContext(nc) as tc:
        with (
            tc.tile_pool(name="persist", bufs=1) as pp,
            tc.tile_pool(name="psum", bufs=7, space="PSUM") as psp,
            tc.tile_pool(name="wpsum", bufs=1, space="PSUM") as wpsp,
        ):
            xt = pp.tile([P, (S // P) * B], dt, tag="xt", name="xt")
            wts = [
                pp.tile(
                    [P, w_ds[i].shape[1]], dt, tag=f"w{i}", name=f"w{i}_sb"
                )
                for i in range(4)
            ]
            bti = (
                pp.tile([P, 11], f32, tag="bias", name="bias_sb")
                if add_bias
                else None
            )
            scr = (
                pp.tile([P, 2], f32, tag="scr", name="scr") if add_bias else None
            )
            acts = [
                [
                    pp.tile([P, B], dt, tag=f"a{li}_{i}", name=f"a{li}_{i}")
                    for i in range(n)
                ]
                for li, n in [(1, H1 // P), (2, H2 // P), (3, D1 // P)]
            ]
            acts.append([pp.tile([P, B], f32, tag="ot", name="ot")])

            # ---- PE warm-up: matmuls on a zeroed scratch tile so the HAM
            # clock gate opens (~3.4us of PE busy) before real data lands.
            if warm > 0:
                wdt = f32 if dt == mybir.dt.float32r else dt
                wsb = pp.tile([P, BT], wdt, tag="wsb", name="wsb")
                nc.gpsimd.memset(wsb[:], 0.0)
                wps = wpsp.tile([P, BT], f32, tag="wps", name="wps")
                lhs = wsb[:, 0:P]
                rhs = wsb[:]
                if dt == mybir.dt.float32r:
                    lhs = lhs.bitcast(dt)
                    rhs = rhs.bitcast(dt)
                for _ in range(warm):
                    nc.tensor.matmul(wps[:], lhs, rhs, start=True, stop=True)

            # ---- input DMAs in compute-need order.
            def dma_x(k, bt):
                sl = slice(k * B + bt * BT, k * B + (bt + 1) * BT)
                nc.sync.dma_start(xt[:, sl], x_d[:, sl])

            def dma_w(li, m=None):
                if m is None:
                    nc.sync.dma_start(wts[li][:], w_ds[li][:])
                else:
                    kc = _LAYERS[li][0]
                    sl = slice(m * kc * P, (m + 1) * kc * P)
                    nc.sync.dma_start(wts[li][:, sl], w_ds[li][:, sl])

            dma_w(0, 0)
            dma_x(0, 0)
            dma_x(1, 0)
            for m in range(1, H1 // P):
                dma_w(0, m)
            dma_x(0, 1)
            dma_x(1, 1)
            dma_w(1)
            dma_x(0, 2)
            dma_x(1, 2)
            dma_w(2)
            dma_x(0, 3)
            dma_x(1, 3)
            dma_w(3)
            if add_bias:
                nc.sync.dma_start(bti[:], b_d[:])
                # advance ACT/DVE engine clocks past the bias DMA so the
                # real post-matmul ops carry a single (PE) wait each — the
                # AC/DVE instruction structs have one wait slot.
                nc.scalar.copy(scr[:, 0:1], bti[:, 0:1])
                nc.vector.tensor_copy(scr[:, 1:2], bti[:, 0:1])

            # ---- the 4-layer chain, batch-tile-outer for a DMA-aligned
            # wavefront.
            xch = [xt[:, k * B : (k + 1) * B] for k in range(S // P)]
            srcs = xch
            for li, (kc, mc, boff, relu) in enumerate(_LAYERS):
                wt = wts[li]
                dsts = acts[li]
                for bt in range(NBT):
                    for m in range(mc):
                        # fixed engine per dst tile: one writer per tile
                        use_act = (li < 3) and (m < mc // 2 or mc == 1)
                        ps = psp.tile([P, BT], f32, tag="ps", name="ps")
                        for k in range(kc):
                            nc.tensor.matmul(
                                ps[:],
                                wt[:, (m * kc + k) * P : (m * kc + k + 1) * P],
                                srcs[k][:, bt * BT : (bt + 1) * BT],
                                start=(k == 0),
                                stop=(k == kc - 1),
                            )
                        dst = dsts[m][:, bt * BT : (bt + 1) * BT]
                        if add_bias:
                            bias_ap = bti[:, boff + m : boff + m + 1]
                            if use_act:
                                func = (
                                    mybir.ActivationFunctionType.Relu
                                    if relu
                                    else mybir.ActivationFunctionType.Identity
                                )
                                nc.scalar.activation(
                                    dst, ps[:], func, bias=bias_ap
                                )
                            elif relu:
                                nc.vector.tensor_scalar(
                                    dst,
                                    ps[:],
                                    bias_ap,
                                    0.0,
                                    mybir.AluOpType.add,
                                    mybir.AluOpType.max,
                                )
                            else:
                                nc.vector.tensor_scalar_add(dst, ps[:], bias_ap)
                        elif use_act:
                            func = (
                                mybir.ActivationFunctionType.Relu
                                if relu
                                else mybir.ActivationFunctionType.Copy
                            )
                            nc.scalar.activation(dst, ps[:], func)
                        elif relu:
                            nc.vector.tensor_scalar_max(dst, ps[:], 0.0)
                        else:
                            nc.vector.tensor_copy(dst, ps[:])
                    if li == 3:
                        nc.sync.dma_start(
                            out_d[:, bt * BT : (bt + 1) * BT],
                            acts[3][0][:, bt * BT : (bt + 1) * BT],
                        )
                srcs = dsts
    nc.compile()
    return nc


_BUILT: dict[tuple, bass.Bass] = {}


def _cfg():
    dt_name = os.environ.get("MADPS_DT", "f32r")
    warm = int(os.environ.get("MADPS_WARM", "6"))
    return dt_name, warm


def _get_nc(dt_name: str, add_bias: bool, warm: int) -> bass.Bass:
    key = (dt_name, add_bias, warm)
    if key not in _BUILT:
        _BUILT[key] = _build(dt_name, add_bias, warm)
    return _BUILT[key]


def _np_dt(dt_name: str):
    if dt_name == "bf16":
        import ml_dtypes

        return ml_dtypes.bfloat16
    return np.float32


def _packw(w: np.ndarray, np_dt) -> np.ndarray:
    """[K, M] -> [128, (K/128)*M], m-chunk-major: col (m*kc + k)*128 + j."""
    k, m = w.shape
    kc, mc = k // P, m // P
    return np.ascontiguousarray(
        w.reshape(kc, P, mc, P).transpose(1, 2, 0, 3).reshape(P, -1).astype(np_dt)
    )


def _prepare(inputs, dt_name):
    """Returns (add_bias, in_maps) for run_bass_kernel_spmd."""
    np_dt = _np_dt(dt_name)

    x = np.asarray(inputs["inputs"], dtype=np.float32)
    sel_s = np.asarray(inputs["laac_shallow"]).reshape(-1).astype(np.int64)
    sel_d = np.asarray(inputs["laac_deep"]).reshape(-1).astype(np.int64)
    Ws1 = np.asarray(inputs["Ws1"], dtype=np.float32)
    Ws2 = np.asarray(inputs["Ws2"], dtype=np.float32)
    Wd1 = np.asarray(inputs["Wd1"], dtype=np.float32)
    Wd2 = np.asarray(inputs["Wd2"], dtype=np.float32)
    bs1 = np.asarray(inputs["bs1"], dtype=np.float32)
    bs2 = np.asarray(inputs["bs2"], dtype=np.float32)
    bd1 = np.asarray(inputs["bd1"], dtype=np.float32)
    bd2 = np.asarray(inputs["bd2"], dtype=np.float32)

    add_bias = any(
        float(np.abs(b).max()) != 0.0 for b in (bs1, bs2, bd1, bd2)
    )

    in_maps = []
    for a in range(A):
        es, ed = int(sel_s[a]), int(sel_d[a])
        xp = np.ascontiguousarray(
            x[a].reshape(B, S // P, P).transpose(2, 1, 0).reshape(P, -1).astype(np_dt)
        )
        m = {
            "x": xp,
            "w1": _packw(Ws1[es], np_dt),
            "w2": _packw(Ws2[es], np_dt),
            "w3": _packw(Wd1[ed], np_dt),
            "w4": _packw(Wd2[ed], np_dt),
        }
        if add_bias:
            bias_cols = np.concatenate([bs1[es], bs2[es], bd1[ed], bd2[ed]])
            m["bias"] = np.ascontiguousarray(
                bias_cols.reshape(11, P).T, dtype=np.float32
            )
        in_maps.append(m)
    return add_bias, in_maps


def kernel(**inputs) -> np.ndarray:
    dt_name, warm = _cfg()
    add_bias, in_maps = _prepare(inputs, dt_name)
    nc = _get_nc(dt_name, add_bias, warm)
    res = run_bass_kernel_spmd(nc, in_maps, list(range(A)))
    out = np.stack([np.asarray(res.results[a]["out"]).T for a in range(A)])
    return np.ascontiguousarray(out.astype(np.float32))
